# revision 10
# baseline (speedup 1.0000x reference)
"""BiAttention kernel for Trainium2 (Bass/Tile), data-parallel over batch on 8 cores.

Math (per batch b):
  att[l,m] = s_in[l] + g[m] + S[l,m]
    S[l,m]  = sum_d inp[l,d]*dot_scale[d]*mem[m,d]
    s_in[l] = sum_d inp[l,d]*w_input[d]
    g[m]    = sum_d mem[m,d]*w_memory[d] + (mask[m]-1)*1e30
  weight_one = softmax_m(att) = softmax_m(S + g)           (s_in cancels)
  output_one = weight_one @ mem
  w2u[l] = exp(max_m att[l,:]) = max_m exp(S+g) * exp(s_in[l])
  output_two = (w2u/sum w2u) @ inp
  out = concat([inp, output_one, inp*output_one, output_two*output_one], -1)

v2 design:
  - S computed transposed (S_T[m,l]) with ONE fp8 DoubleRow matmul per m-tile
    (K=256 contraction in one instruction): inT8 = fp8(32*dsc*inp^T),
    memT8 = fp8(4*mem^T), exp scale = 1/128.
  - exp(g) is folded into mm2's rhs (mem_sb *= exp(g[m]) per partition) and into
    the running max (scalar_tensor_tensor mult+max), NOT into the exp bias.
    This makes every exp identical -> one paired exp per 2 m-tiles (N=1024).
  - Softmax denominators from a ones-column in mem_sb (scaled by exp(g), which
    cancels in the normalization).
  - s_in on the PE: rhs column w_input[d]/(32*dsc[d]) (clamped to fp8 range)
    against inT8.
  - output_two accumulated per-quarter into a borrowed PSUM slot, folded into
    an SBUF accumulator; final normalize/broadcast/multiply in the epilogue.
"""

import threading

import numpy as np

import concourse.bacc as bacc
import concourse.bass as bass
import concourse.mybir as mybir
import concourse.tile as tile
from concourse.masks import make_identity

F32 = mybir.dt.float32
BF16 = mybir.dt.bfloat16
FP8 = mybir.dt.float8e4
AF = mybir.ActivationFunctionType
ALU = mybir.AluOpType
AX = mybir.AxisListType
DR = mybir.MatmulPerfMode.DoubleRow

B, L, M, D = 8, 2048, 2048, 256
P = 128
LT = L // P          # 16 l-tiles
MT = M // P          # 16 m-tiles
KD = D // P          # 2 contraction tiles
NQ = 4               # l-quarters (PSUM-accumulator constraint)
QW = L // NQ         # 512
QT = QW // P         # 4 l-tiles per quarter
NPAIR = MT // 2      # 8 m-tile pairs per quarter
NEG_BIG = 1.0e30
FP8MAX = 448.0


def build_nc():
    nc = bacc.Bacc(
        "TRN2", target_bir_lowering=False, debug=False, num_devices=8
    )

    inp_d = nc.dram_tensor("input", [L, D], F32, kind="ExternalInput").ap()
    mem_d = nc.dram_tensor("memory", [M, D], F32, kind="ExternalInput").ap()
    mask_d = nc.dram_tensor("mask", [M], F32, kind="ExternalInput").ap()
    w_in_d = nc.dram_tensor("w_input", [D], F32, kind="ExternalInput").ap()
    w_mem_d = nc.dram_tensor("w_memory", [D], F32, kind="ExternalInput").ap()
    dsc_d = nc.dram_tensor("dot_scale", [D], F32, kind="ExternalInput").ap()
    out_d = nc.dram_tensor("out", [L, 4 * D], F32, kind="ExternalOutput").ap()

    inp_r = inp_d.rearrange("(t p) d -> p t d", p=P)      # [128,16,256]
    mem_r = mem_d.rearrange("(t p) d -> p t d", p=P)      # [128,16,256]
    mask_r = mask_d.rearrange("(t p) -> t p", p=P)        # [16,128]
    out_r = out_d.rearrange("(t p) c -> p t c", p=P)      # [128,16,1024]

    with tile.TileContext(nc) as tc:
        with (
            tc.tile_pool(name="consts", bufs=1) as cp,
            tc.tile_pool(name="ptiles", bufs=4) as pp,
            tc.tile_pool(name="stage", bufs=4) as sp,
            tc.tile_pool(name="rp", bufs=4) as rp,
        ):
            # ---------------- persistent SBUF ----------------
            ident_b = cp.tile([P, P], BF16)
            ident_f = cp.tile([P, P], F32)
            in_f32 = cp.tile([P, LT, D], F32)       # natural input (f32, load staging)
            inp_bf = cp.tile([P, LT, D], BF16)      # natural input (bf16)
            mem_sb = cp.tile([P, MT, D + 1], BF16)  # natural memory + ones col
            inT8 = cp.tile([P, KD, L], FP8)         # fp8(32*dsc*inp^T)
            memT8 = cp.tile([P, KD, M], FP8)        # fp8(4*mem^T)
            maxacc = cp.tile([P, L], BF16)          # running max of exp(S+g)
            out1_sb = cp.tile([P, LT, D], F32)      # normalized output_one
            vpad = cp.tile([P, P], F32)             # dsc 0:2, w_mem 2:4, w_in 4:6, mask 16:32
            dsc32 = cp.tile([P, KD], F32)
            w_mem8 = cp.tile([P, KD, 1], FP8)
            w_in8 = cp.tile([P, KD, 1], FP8)
            gtmp = cp.tile([P, MT], F32)
            g_col = cp.tile([P, MT], F32)
            expg = cp.tile([P, MT], F32)
            s_in_sb = cp.tile([P, LT], F32)
            exp_si = cp.tile([P, LT], F32)
            rowmax = cp.tile([P, LT], F32)
            w2u = cp.tile([P, LT], F32)
            w2u_bf = cp.tile([P, LT], BF16)
            o2acc = cp.tile([1, D], F32)
            o2n = cp.tile([1, D], F32)
            o2b_bf = cp.tile([P, D], BF16)
            ones_row = cp.tile([1, P], F32)
            ones_col = cp.tile([P, 1], F32)
            w2s = cp.tile([P, 1], F32)
            rtot = cp.tile([1, 1], F32)
            warm = cp.tile([P, 1], F32)

            # ---------------- prologue: loads first ----------------
            nc.vector.memset(ones_col[:], 1.0)
            nc.vector.memset(ones_row[:], 1.0)
            # touch Exp early so the ACT table load is off the critical path
            nc.scalar.activation(out=warm[:], in_=ones_col[:], func=AF.Exp)

            # small params + mask (scalar queue), all into vpad rows
            nc.scalar.dma_start(
                out=vpad[0:KD, :], in_=dsc_d.rearrange("(k p) -> k p", p=P)
            )
            nc.scalar.dma_start(
                out=vpad[KD : 2 * KD, :], in_=w_mem_d.rearrange("(k p) -> k p", p=P)
            )
            nc.scalar.dma_start(
                out=vpad[2 * KD : 3 * KD, :], in_=w_in_d.rearrange("(k p) -> k p", p=P)
            )
            nc.scalar.dma_start(out=vpad[16:32, :], in_=mask_r)

            # identities first: make_identity runs on the gpsimd engine and
            # must not queue behind the software-DGE memory loads below
            make_identity(nc, ident_b)
            make_identity(nc, ident_f)

            # big loads: input split across sync+scalar HWDGE queues,
            # memory on gpsimd (the only engine that can cast in the DMA)
            for c in range(8):
                q = nc.sync if c % 2 == 0 else nc.scalar
                q.dma_start(
                    out=in_f32[:, c * 2 : (c + 1) * 2, :],
                    in_=inp_r[:, c * 2 : (c + 1) * 2, :],
                )
                nc.gpsimd.dma_start(
                    out=mem_sb[:, c * 2 : (c + 1) * 2, 0:D],
                    in_=mem_r[:, c * 2 : (c + 1) * 2, :],
                )
            # block 0 of the output is the input verbatim: DRAM -> DRAM copy.
            # Queued after the loads; runs during the main loop.
            nc.sync.dma_start(
                out=out_d[0 : L // 2, 0:D], in_=inp_d[0 : L // 2, :]
            )
            nc.scalar.dma_start(
                out=out_d[L // 2 : L, 0:D], in_=inp_d[L // 2 : L, :]
            )
            nc.vector.memset(mem_sb[:, :, D : D + 1], 1.0)
            nc.vector.memset(maxacc[:], 0.0)
            nc.vector.memset(o2acc[:], 0.0)

            with tc.tile_pool(name="psT", bufs=4, space="PSUM") as psT:
                # ---------------- small params ----------------
                pv = psT.tile([P, P], F32, tag="trx")
                nc.tensor.transpose(pv[:], vpad[:], ident_f[:])
                nc.vector.tensor_scalar(
                    out=dsc32[:], in0=pv[:, 0:KD], scalar1=32.0, scalar2=None,
                    op0=ALU.mult,
                )
                nc.vector.tensor_scalar(
                    out=w_mem8[:, :, 0], in0=pv[:, KD : 2 * KD],
                    scalar1=16.0, scalar2=None, op0=ALU.mult,
                )
                rcp = rp.tile([P, KD], F32)
                nc.vector.reciprocal(rcp[:], dsc32[:])
                wint = rp.tile([P, KD], F32)
                nc.vector.tensor_mul(wint[:], pv[:, 2 * KD : 3 * KD], rcp[:])
                nc.vector.tensor_scalar(
                    out=w_in8[:, :, 0], in0=wint[:],
                    scalar1=FP8MAX, scalar2=-FP8MAX, op0=ALU.min, op1=ALU.max,
                )
                nc.vector.tensor_scalar(
                    out=gtmp[:], in0=pv[:, 16:32], scalar1=1.0, scalar2=NEG_BIG,
                    op0=ALU.subtract, op1=ALU.mult,
                )

                dots = psT.tile([P, 2 * MT], F32, tag="dots", bufs=1)
                mdp = dots[:, 0:MT]
                sinp = dots[:, MT : 2 * MT]

                # ---------------- memory batches ----------------
                def memory_batch(bi):
                    t0 = 2 * bi
                    trx = psT.tile([P, 512], BF16, name=f"trm{bi}", tag="trx")
                    j = 0
                    for k in range(KD):
                        for t in (t0, t0 + 1):
                            nc.tensor.transpose(
                                trx[:, j * P : (j + 1) * P],
                                mem_sb[:, t, k * P : (k + 1) * P],
                                ident_b,
                            )
                            j += 1
                    # fp8 cast with uniform *4 scale (ACT is idle in prologue)
                    nc.scalar.activation(
                        out=memT8[:, :, t0 * P : (t0 + 2) * P],
                        in_=trx.rearrange("p (k f) -> p k f", k=KD),
                        func=AF.Copy,
                        scale=4.0,
                    )
                    # memory_dot for the two m-tiles (one DoubleRow MM each)
                    for t in (t0, t0 + 1):
                        nc.tensor.matmul(
                            mdp[:, t : t + 1],
                            lhsT=memT8[:, :, t * P : (t + 1) * P],
                            rhs=w_mem8[:],
                            start=(t == 0),
                            stop=(t == MT - 1),
                            perf_mode=DR,
                        )

                def input_batch(bi):
                    t0 = 2 * bi
                    # cast this batch's input chunk f32 -> bf16 just in time so
                    # the DVE's in-order queue isn't stuck behind later loads
                    nc.vector.tensor_copy(
                        inp_bf[:, t0 : t0 + 2, :].rearrange("p t d -> p (t d)"),
                        in_f32[:, t0 : t0 + 2, :].rearrange("p t d -> p (t d)"),
                    )
                    trx = psT.tile([P, 512], BF16, name=f"tri{bi}", tag="trx")
                    j = 0
                    for k in range(KD):
                        for t in (t0, t0 + 1):
                            nc.tensor.transpose(
                                trx[:, j * P : (j + 1) * P],
                                inp_bf[:, t, k * P : (k + 1) * P],
                                ident_b,
                            )
                            j += 1
                    for k in range(KD):
                        nc.vector.tensor_scalar(
                            out=inT8[:, k, t0 * P : (t0 + 2) * P],
                            in0=trx[:, k * 2 * P : (k + 1) * 2 * P],
                            scalar1=dsc32[:, k : k + 1], scalar2=None,
                            op0=ALU.mult,
                        )

                for bi in range(8):
                    input_batch(bi)
                    memory_batch(bi)

                # g = mdp/64 + mask term; expg = exp(g); fold into mem_sb
                nc.vector.scalar_tensor_tensor(
                    out=g_col[:], in0=mdp[:], scalar=1.0 / 64.0, in1=gtmp[:],
                    op0=ALU.mult, op1=ALU.add,
                )
                nc.scalar.activation(out=expg[:], in_=g_col[:], func=AF.Exp)
                for t in range(MT):
                    nc.vector.tensor_scalar(
                        out=mem_sb[:, t, :], in0=mem_sb[:, t, :],
                        scalar1=expg[:, t : t + 1], scalar2=None, op0=ALU.mult,
                    )

                # s_in on the PE (fp8, DoubleRow, N=1 each)
                for tg in range(LT):
                    nc.tensor.matmul(
                        sinp[:, tg : tg + 1],
                        lhsT=inT8[:, :, tg * P : (tg + 1) * P],
                        rhs=w_in8[:],
                        start=(tg == 0),
                        stop=(tg == LT - 1),
                        perf_mode=DR,
                    )
                nc.vector.tensor_copy(s_in_sb[:], sinp[:])
                nc.scalar.activation(out=exp_si[:], in_=s_in_sb[:], func=AF.Exp)

            # ---------------- main loop ----------------
            with (
                tc.tile_pool(name="psM", bufs=2, space="PSUM") as psM,
                tc.tile_pool(name="psA", bufs=4, space="PSUM") as psA,
            ):
                def emit_mm1_pair(gp, ps):
                    q, p = gp // NPAIR, gp % NPAIR
                    for i, t in enumerate((2 * p, 2 * p + 1)):
                        nc.tensor.matmul(
                            ps[:, i * QW : (i + 1) * QW],
                            lhsT=memT8[:, :, t * P : (t + 1) * P],
                            rhs=inT8[:, :, q * QW : (q + 1) * QW],
                            start=True,
                            stop=True,
                            perf_mode=DR,
                        )

                def emit_o2_partial(qd):
                    o2q = psM.tile([1, D], F32, name=f"o2q{qd}", tag="m")
                    for i in range(QT):
                        tg = qd * QT + i
                        nc.tensor.matmul(
                            o2q[:],
                            lhsT=w2u_bf[:, tg : tg + 1],
                            rhs=inp_bf[:, tg, :],
                            start=(i == 0),
                            stop=(i == QT - 1),
                        )
                    nc.vector.tensor_add(o2acc[:], o2acc[:], o2q[:])

                def quarter_end(q):
                    # row max over partitions via PE transpose + free-dim reduce
                    trp = psM.tile([P, QW], BF16, name=f"trp{q}", tag="m")
                    for lt in range(QT):
                        nc.tensor.transpose(
                            trp[:, lt * P : (lt + 1) * P],
                            maxacc[:, (q * QT + lt) * P : (q * QT + lt + 1) * P],
                            ident_b,
                        )
                    nc.vector.reduce_max(
                        rowmax[:, q * QT : (q + 1) * QT],
                        trp.rearrange("p (lt x) -> p lt x", x=P),
                        axis=AX.X,
                    )
                    nc.vector.tensor_mul(
                        w2u[:, q * QT : (q + 1) * QT],
                        rowmax[:, q * QT : (q + 1) * QT],
                        exp_si[:, q * QT : (q + 1) * QT],
                    )
                    nc.vector.tensor_copy(
                        w2u_bf[:, q * QT : (q + 1) * QT],
                        w2u[:, q * QT : (q + 1) * QT],
                    )
                    # normalize output_one; write blocks 1 and 2
                    for lt in range(QT):
                        tg = q * QT + lt
                        r = rp.tile([P, 1], F32, name=f"r{tg}", tag="r")
                        nc.vector.reciprocal(r[:], accs[lt][:, D : D + 1])
                        nc.scalar.activation(
                            out=out1_sb[:, tg, :], in_=accs[lt][:, 0:D],
                            func=AF.Copy, scale=r[:],
                        )
                        st = sp.tile([P, D], F32, name=f"st{tg}", tag="st")
                        nc.vector.tensor_mul(st[:], inp_bf[:, tg, :], out1_sb[:, tg, :])
                        nc.sync.dma_start(out=out_r[:, tg, 2 * D : 3 * D], in_=st[:])
                    nc.sync.dma_start(
                        out=out_r[:, q * QT : (q + 1) * QT, D : 2 * D],
                        in_=out1_sb[:, q * QT : (q + 1) * QT, :],
                    )

                accs = None
                ps_next = psM.tile([P, 2 * QW], F32, tag="m", name="ps_g0")
                emit_mm1_pair(0, ps_next)
                for gp in range(NQ * NPAIR):
                    q, p = gp // NPAIR, gp % NPAIR
                    if p == 0:
                        accs = [
                            psA.tile([P, D + 1], F32, tag="acc", name=f"acc_q{q}_{i}")
                            for i in range(QT)
                        ]
                    ps = ps_next
                    if gp + 1 < NQ * NPAIR:
                        ps_next = psM.tile(
                            [P, 2 * QW], F32, tag="m", name=f"ps_g{gp + 1}"
                        )
                        emit_mm1_pair(gp + 1, ps_next)
                    pt = pp.tile([P, 2 * QW], BF16)
                    nc.scalar.activation(
                        out=pt[:], in_=ps[:], func=AF.Exp, scale=1.0 / 128.0
                    )
                    msl = maxacc[:, q * QW : (q + 1) * QW]
                    for i, t in ((0, 2 * p), (1, 2 * p + 1)):
                        pte = rp.tile([P, QW], BF16, name=f"pte{gp}_{i}", tag="pte")
                        nc.vector.tensor_scalar(
                            out=pte[:], in0=pt[:, i * QW : (i + 1) * QW],
                            scalar1=expg[:, t : t + 1], scalar2=None, op0=ALU.mult,
                        )
                        nc.vector.tensor_max(msl, msl, pte[:])
                    if p == 1 and q > 0:
                        emit_o2_partial(q - 1)
                    for i, t in enumerate((2 * p, 2 * p + 1)):
                        for lt in range(QT):
                            nc.tensor.matmul(
                                accs[lt][:],
                                lhsT=pt[:, i * QW + lt * P : i * QW + (lt + 1) * P],
                                rhs=mem_sb[:, t, :],
                                start=(p == 0 and i == 0),
                                stop=(p == NPAIR - 1 and i == 1),
                            )
                    if p == NPAIR - 1:
                        quarter_end(q)

                # ---------------- weight_two tail ----------------
                emit_o2_partial(NQ - 1)
                nc.vector.reduce_sum(w2s[:], w2u[:], axis=AX.X)
                totp = psM.tile([1, 1], F32, tag="m")
                nc.tensor.matmul(
                    totp[:], lhsT=w2s[:], rhs=ones_col[:], start=True, stop=True
                )
                nc.vector.reciprocal(rtot[:], totp[:])
                nc.vector.tensor_scalar(
                    out=o2n[:], in0=o2acc[:], scalar1=rtot[:], scalar2=None,
                    op0=ALU.mult,
                )
                o2bp = psM.tile([P, D], F32, tag="m")
                nc.tensor.matmul(
                    o2bp[:], lhsT=ones_row[:], rhs=o2n[:], start=True, stop=True
                )
                nc.scalar.activation(out=o2b_bf[:], in_=o2bp[:], func=AF.Copy)

                o2bc = o2b_bf.rearrange("p (o d) -> p o d", o=1).broadcast_to(
                    [P, QT, D]
                )
                for tg in range(12, LT):
                    o4s = sp.tile([P, D], F32, name=f"o4s_{tg}", tag="o4s", bufs=4)
                    nc.gpsimd.tensor_mul(o4s[:], o2b_bf[:], out1_sb[:, tg, :])
                    q = nc.sync if tg % 2 == 0 else nc.scalar
                    q.dma_start(out=out_r[:, tg, 3 * D : 4 * D], in_=o4s[:])
                for c in range(3):
                    o4 = sp.tile([P, QT, D], F32, name=f"o4_{c}", tag="o4", bufs=3)
                    nc.vector.tensor_mul(
                        o4[:], o2bc, out1_sb[:, c * QT : (c + 1) * QT, :]
                    )
                    q = nc.scalar if c % 2 == 0 else nc.sync
                    q.dma_start(
                        out=out_r[:, c * QT : (c + 1) * QT, 3 * D : 4 * D], in_=o4[:]
                    )

    nc.compile()
    return nc


_CACHE = threading.local()


def _get_nc():
    nc = getattr(_CACHE, "nc", None)
    if nc is None:
        nc = build_nc()
        _CACHE.nc = nc
    return nc


def make_in_maps(input, memory, mask, w_input, w_memory, dot_scale):
    input = np.ascontiguousarray(np.asarray(input, dtype=np.float32))
    memory = np.ascontiguousarray(np.asarray(memory, dtype=np.float32))
    mask = np.ascontiguousarray(np.asarray(mask, dtype=np.float32))
    w_input = np.ascontiguousarray(np.asarray(w_input, dtype=np.float32))
    w_memory = np.ascontiguousarray(np.asarray(w_memory, dtype=np.float32))
    dot_scale = np.ascontiguousarray(np.asarray(dot_scale, dtype=np.float32))
    return [
        {
            "input": input[b],
            "memory": memory[b],
            "mask": mask[b],
            "w_input": w_input,
            "w_memory": w_memory,
            "dot_scale": dot_scale,
        }
        for b in range(B)
    ]


def _run_once(nc, in_maps):
    from concourse.bass_utils import run_bass_kernel_spmd

    res = run_bass_kernel_spmd(nc, in_maps, core_ids=list(range(B)))
    return np.stack([res.results[b]["out"] for b in range(B)], axis=0)


def kernel(input, memory, mask, w_input, w_memory, dot_scale):
    nc = _get_nc()
    in_maps = make_in_maps(input, memory, mask, w_input, w_memory, dot_scale)
    # The kernel is deterministic; rarely a core returns corrupted data after
    # an earlier device fault.  Run twice and require agreement.
    out = _run_once(nc, in_maps)
    for _ in range(3):
        out2 = _run_once(nc, in_maps)
        if np.array_equal(out, out2):
            return out
        out = out2
    return out
